# revision 23
# baseline (speedup 1.0000x reference)
"""MBConv block with MoE routing on 8 trn2 cores — fp8 DoubleRow everywhere.

Sharding: pure data parallel — batch 64 split 8 samples per core; all weights
replicated. Device kernel computes routing, expert-weight aggregation, expand
conv (fp8 DoubleRow over channel halves + a ones-channel carrying bn1's bias),
per-sample depthwise conv as fp8e4 DoubleRow diagonal matmuls (2 taps per
matmul, even-stride pairs, plus a bias pair against a ones strip carrying
bn2's bias), squeeze-excitation, pointwise projection (fp8 DoubleRow over
chunk pairs), bn3 fold and residual.

All BN folds are absorbed into weights (a1 -> expand cols, a2 -> depthwise
kernels, b1/b2 via ones channels, b3 pre-added to x on the host with an exact
rb1 correction) so every activation instruction uses only immediate
scale/bias — per-partition AP scale/bias activations are ~3x slower on hw.

Diag slabs for the depthwise stationary are built 4x faster than a plain fp8
elementwise pass: the fp8-encoded kernel bytes are lane-shifted into uint32
(kern32 = byte * 2^(8*(p%4))) and multiplied against a uint32 0/1 mask
through a bitcast view of the fp8 slab, so DVE writes 832 u32 elems instead
of 3328 fp8 elems per (sample, chunk). Integer products are exact in f32.

Expert aggregation is per-sample on DVE, interleaved so each sample's
kernel path overlaps the previous sample's depthwise work; the depthwise
silu is one fused activation per (sample, chunk) across both row-halves
reading two psum banks; the SE chain is split in three parts interleaved
with the next sample's depthwise matmuls so its cross-engine latency hides
under PE work. Per-sample x slices prefetch across bench-loop iterations.

Numerics: expand weights x8, depthwise kernels x64, pointwise weights x64 —
each pre-scale undone by the activation's immediate scale or the bn3 fold.

Self-contained: hardcodes all shapes; host side only reshapes/prepacks weights.
"""

import os
import sys
import time

for _p in ("/opt/trn_rl_repo", os.path.expanduser("~/.axon_site/_ro/trn_rl_repo")):
    if os.path.isdir(_p) and _p not in sys.path:
        sys.path.insert(0, _p)

import contextlib

import numpy as np

import concourse.bacc as bacc
import concourse.bass as bass
import concourse.tile as tile
from concourse import mybir

F32 = mybir.dt.float32
U32 = mybir.dt.uint32
U8 = mybir.dt.uint8
FP8 = mybir.dt.float8e4
AF = mybir.ActivationFunctionType
ALU = mybir.AluOpType
AX = mybir.AxisListType
DR = mybir.MatmulPerfMode.DoubleRow

# dims (must match the problem spec)
B, CIN, H, W = 64, 96, 28, 28
NCORES = 8
BL = B // NCORES          # 8 samples per core
E = 4
HID = 576
KK = 5
T = KK * KK               # 25 taps
TP = 26                   # taps padded with one zero tap for DoubleRow pairing
RED = 24                  # SE reduced dim
RHID = 24                 # routing hidden
COUT = 96
EPS = 1e-3
HW = H * W                # 784
NG = 5                    # ceil(576/128) channel chunks
GP = 128
HIDP = NG * GP            # 640 padded
PW = 30                   # padded row stride (28 cols + 2 halo)
NH = 2                    # output row halves (14 rows each)
RH = H // NH              # 14
NF = RH * W               # 392 free elems per half
XPW = 968                 # padded x1 tile width (62 head + 30 rows x 30 + DR tail)
KSCALE = 64.0             # fp8 kernel pre-scale, undone by bn2 act imm scale
ESCALE = 8.0              # fp8 expand pre-scale, undone by bn1 act imm scale
PSCALE = 64.0             # fp8 pointwise pre-scale, folded out via a3
CH = CIN // 2 + 1         # 49: expand DR contraction half + ones channel
NSLOT = 5                 # xp slot depth (expand runs NSLOT samples ahead)
NOUT = 3                  # out2 slot depth

# DoubleRow tap pairs (t0, t1): the rhs plane stride (byte delta between the
# two shifted windows) must be EVEN — odd strides crash the DGE. Within-row
# pairs are taken 2 apart (stride 2), the kw=4 column pairs vertically
# (stride 32), tap 24 pairs with zero tap 25, and the bias pair (26, 27)
# reads a ones strip against diag(64*b2) + a zero plane.
DW_PAIRS = (
    [(5 * r, 5 * r + 2) for r in range(KK)]
    + [(5 * r + 1, 5 * r + 3) for r in range(KK)]
    + [(4, 9), (14, 19), (24, 25)]
)


def _tap_off(t):
    kh, kw = divmod(t, KK)
    return PW * kh + kw


def _build_program(reps=1, ablate=()):
    nc = bacc.Bacc(None, target_bir_lowering=False)

    dt = lambda name, shape: nc.dram_tensor(name, shape, F32, kind="ExternalInput")
    x_d = dt("x", [CIN, BL, HW])
    x8_d = nc.dram_tensor("x8", [CH, 2, BL, HW], FP8, kind="ExternalInput")
    exp8_d = nc.dram_tensor("exp8", [CH, 2, HIDP], FP8, kind="ExternalInput")
    ident32_d = nc.dram_tensor("ident32", [GP, TP * 32], U32,
                               kind="ExternalInput")
    lane32_d = nc.dram_tensor("lane32", [GP, 1], F32, kind="ExternalInput")
    b2_d = dt("b2", [GP, NG])
    a3_d = dt("a3", [COUT, 1])
    dwT_d = dt("dwT", [GP, E, NG, T])
    pwT_d = dt("pwT", [GP, E, NG, COUT])
    sw1_d = dt("sw1", [GP, NG, RED])
    sw2b_d = dt("sw2b", [RED, NG, GP])
    b2se_d = dt("b2se", [GP, NG])
    rw1_d = dt("rw1", [CIN, RHID])
    rb1_d = dt("rb1", [RHID, 1])
    rw2_d = dt("rw2", [RHID, E])
    rb2_d = dt("rb2", [BL, E])
    sb1_d = dt("sb1", [RED, 1])
    y_d = nc.dram_tensor("y", [BL, COUT, HW], F32, kind="ExternalOutput")

    with tile.TileContext(nc) as tc:
        with (
            tc.tile_pool(name="consts", bufs=1) as cp,
            tc.tile_pool(name="dram", bufs=1, space="DRAM") as dp,
            tc.tile_pool(name="xpad", bufs=1) as xpp,
            tc.tile_pool(name="out2", bufs=1) as o2p,
            tc.tile_pool(name="diag", bufs=15) as dgp,
            tc.tile_pool(name="wscp", bufs=3) as wsp,
            tc.tile_pool(name="outb", bufs=3) as obp,
            tc.tile_pool(name="small", bufs=3) as smp,
            tc.tile_pool(name="ppex", bufs=2, space="PSUM") as ppex,
            tc.tile_pool(name="pdw", bufs=2, space="PSUM") as pdwp,
            tc.tile_pool(name="psepw", bufs=2, space="PSUM") as psepw,
        ):
            # ---- persistent consts ----
            x_sb = cp.tile([CIN, BL, HW], F32, tag="x_sb")
            x8 = cp.tile([CH, 2, BL, HW], FP8, tag="x8")
            exp8 = cp.tile([CH, 2, HIDP], FP8, tag="exp8")
            b2 = cp.tile([GP, NG], F32, tag="b2")
            a3 = cp.tile([COUT, 1], F32, tag="a3")
            dwT = cp.tile([GP, E, NG, T], F32, tag="dwT")
            pwT = cp.tile([GP, E, NG, COUT], F32, tag="pwT")
            sw1 = cp.tile([GP, NG, RED], F32, tag="sw1")
            sw2b = cp.tile([RED, NG, GP], F32, tag="sw2b")
            b2se = cp.tile([GP, NG], F32, tag="b2se")
            rw1 = cp.tile([CIN, RHID], F32, tag="rw1")
            rb1 = cp.tile([RHID, 1], F32, tag="rb1")
            rw2 = cp.tile([RHID, E], F32, tag="rw2")
            rb2 = cp.tile([BL, E], F32, tag="rb2")
            sb1 = cp.tile([RED, 1], F32, tag="sb1")
            ident32 = cp.tile([GP, TP * 32], U32, tag="ident32")
            lane32 = cp.tile([GP, 1], F32, tag="lane32")
            kern = cp.tile([GP, NG, BL, TP], F32, tag="kern")
            kern8 = cp.tile([GP, NG, BL, TP], FP8, tag="kern8")
            kern32 = cp.tile([GP, NG, BL, TP], U32, tag="kern32")
            pwag = cp.tile([GP, BL, NG, COUT], F32, tag="pwag")
            rw_bc = cp.tile([GP, BL * E], F32, tag="rw_bc")

            # ---- padded x1 tiles: 2 slots x NG chunks, fp8, zeroed once ----
            xp_t = [
                [xpp.tile([GP, XPW], FP8, tag=f"xp{s}g{g}", name=f"xp{s}g{g}")
                 for g in range(NG)]
                for s in range(NSLOT)
            ]
            for s in range(NSLOT):
                for g in range(NG):
                    nc.gpsimd.memset(xp_t[s][g][:], 0.0)
            # zero tap T stays zero forever (kern8/kern32 inherit it)
            nc.gpsimd.memset(kern[:, :, :, T : T + 1], 0.0)

            out2_t = [o2p.tile([GP, NG, HW], FP8, tag=f"o2{s}", name=f"o2{s}")
                      for s in range(NOUT)]
            s_parts = [cp.tile([GP, NG], F32, tag=f"sp{b}", name=f"sp{b}")
                       for b in range(BL)]

            def build_diag(dg, b, g):
                # dg[:, t, :] = diag(64 * kern[:, g, b, t]) via the u32 trick:
                # kern32 holds the fp8 byte of 64*kern lane-shifted to p%4;
                # multiply against the 0/1 u32 mask through a bitcast view.
                dg32 = dg[:].bitcast(U32)
                k_ap = kern32[:, g, b, :]
                k_b = bass.AP(tensor=k_ap.tensor, offset=k_ap.offset,
                              ap=[k_ap.ap[0], [1, TP], [0, 32]])
                nc.vector.tensor_mul(dg32, k_b, ident32[:])

            # SE chain + pointwise for sample b, split in three parts that
            # interleave with the next sample's depthwise matmuls so the
            # cross-engine chain latency (pz->zt->psc->ut->th->wsc->po) hides
            # under PE work instead of stalling it.
            se_state = {}

            def emit_se1(b):
                pz = psepw.tile([RED, 1], F32, tag="se", name="pz")
                for g in range(NG):
                    nc.tensor.matmul(pz[:], sw1[:, g],
                                     s_parts[b][:, g : g + 1],
                                     start=(g == 0), stop=(g == NG - 1))
                zt = smp.tile([RED, 1], F32, tag="zt", name="zt")
                nc.scalar.activation(zt[:], pz[:], AF.Silu, bias=sb1[:],
                                     scale=1.0)
                se_state["zt"] = zt

            def emit_se2(b):
                zt = se_state.pop("zt")
                psc = psepw.tile([GP, NG], F32, tag="se", name="psc")
                for g in range(NG):
                    nc.tensor.matmul(psc[:, g : g + 1], sw2b[:, g], zt[:],
                                     start=True, stop=True)
                # sigmoid via tanh (stays in the silu ACT table set):
                # sigmoid(p + b) = 0.5 + 0.5*tanh(0.5*p + 0.5*b); b2se
                # pre-halved; the remaining 0.5+0.5*th is folded as
                # wsc = pwag*(1 + th) with the extra 0.5 absorbed into a3.
                ut = smp.tile([GP, NG], F32, tag="ut", name="ut")
                nc.vector.scalar_tensor_tensor(ut[:], psc[:], 0.5, b2se[:],
                                               op0=ALU.mult, op1=ALU.add)
                th = smp.tile([GP, NG], F32, tag="th", name="th")
                nc.scalar.activation(th[:], ut[:], AF.Tanh)
                wsc = wsp.tile([GP, NG, COUT], FP8, tag="wsc", name="wsc")
                for g in range(NG):
                    nc.vector.scalar_tensor_tensor(
                        wsc[:, g], pwag[:, b, g], th[:, g : g + 1],
                        pwag[:, b, g], op0=ALU.mult, op1=ALU.add)
                se_state["wsc"] = wsc

            def emit_se3(b, slot):
                wsc = se_state.pop("wsc")
                # pointwise projection: 2 DoubleRow chunk-pairs + 1 plain fp8,
                # stationaries shared across the two row-halves
                po = [psepw.tile([COUT, 512], F32, tag="se", name=f"po{nh}")
                      for nh in range(NH)]
                wsc_ap = wsc[:]
                o2_ap = out2_t[slot][:]
                for gp_i in range(2):
                    g0 = 2 * gp_i
                    lhsT = bass.AP(
                        tensor=wsc_ap.tensor,
                        offset=wsc_ap.offset + g0 * COUT,
                        ap=[wsc_ap.ap[0], [COUT, 2], [1, COUT]])
                    for nh in range(NH):
                        rhs = bass.AP(
                            tensor=o2_ap.tensor,
                            offset=o2_ap.offset + g0 * HW + nh * NF,
                            ap=[o2_ap.ap[0], [HW, 2], [1, NF]])
                        nc.tensor.matmul(po[nh][:, :NF], lhsT, rhs,
                                         start=(gp_i == 0), stop=False,
                                         perf_mode=DR)
                for nh in range(NH):
                    nc.tensor.matmul(
                        po[nh][:, :NF], wsc[:, NG - 1],
                        out2_t[slot][:, NG - 1, nh * NF : (nh + 1) * NF],
                        start=False, stop=True)
                ob = obp.tile([COUT, HW], F32, tag="ob", name="ob")
                for nh in range(NH):
                    nc.vector.scalar_tensor_tensor(
                        ob[:, nh * NF : (nh + 1) * NF], po[nh][:, :NF],
                        a3[:], x_sb[:, b, nh * NF : (nh + 1) * NF],
                        op0=ALU.mult, op1=ALU.add)
                nc.sync.dma_start(y_d[b], ob[:])

            def emit_body():
                # per-sample x slices: slice b's reload only waits on sample
                # b's residual read, so iteration i+1's loads prefetch while
                # iteration i is still running
                for b in range(BL):
                    nc.sync.dma_start(x_sb[:, b], x_d[:, b])
                nc.sync.dma_start(x8[:], x8_d[:])
                for t_sb, t_d in [
                    (exp8, exp8_d), (b2, b2_d), (a3, a3_d),
                    (dwT, dwT_d), (pwT, pwT_d), (sw1, sw1_d), (sw2b, sw2b_d),
                    (b2se, b2se_d), (rw1, rw1_d), (rb1, rb1_d), (rw2, rw2_d),
                    (rb2, rb2_d), (sb1, sb1_d), (ident32, ident32_d),
                    (lane32, lane32_d),
                ]:
                    nc.sync.dma_start(t_sb[:], t_d[:])

                # routing: pool -> MLP -> softmax (samples on partitions).
                # exp(v) for v<=0 computed as (1+t)/(1-t) with t=tanh(v/2) so
                # the act table set never leaves the silu family.
                xsum = cp.tile([CIN, BL], F32, tag="xsum", name="xsum")
                nc.vector.tensor_reduce(xsum[:], x_sb[:], axis=AX.X, op=ALU.add)
                ph1 = psepw.tile([RHID, BL], F32, tag="se", name="ph1")
                nc.tensor.matmul(ph1[:], rw1[:], xsum[:], start=True, stop=True)
                hdn = cp.tile([RHID, BL], F32, tag="hdn", name="hdn")
                nc.scalar.activation(hdn[:], ph1[:], AF.Relu, bias=rb1[:], scale=1.0)
                pl2 = psepw.tile([BL, E], F32, tag="se", name="pl2")
                nc.tensor.matmul(pl2[:], hdn[:], rw2[:], start=True, stop=True)
                lt = cp.tile([BL, E], F32, tag="lt", name="lt")
                nc.vector.tensor_add(lt[:], pl2[:], rb2[:])
                mx = cp.tile([BL, 1], F32, tag="mx", name="mx")
                nc.vector.reduce_max(mx[:], lt[:], axis=AX.X)
                nc.vector.tensor_scalar_sub(lt[:], lt[:], mx[:])
                th = cp.tile([BL, E], F32, tag="th", name="th")
                nc.scalar.activation(th[:], lt[:], AF.Tanh, bias=0.0, scale=0.5)
                el = cp.tile([BL, E], F32, tag="el", name="el")
                den = cp.tile([BL, E], F32, tag="den", name="den")
                nc.vector.tensor_scalar(el[:], th[:], 1.0, 1.0,
                                        op0=ALU.mult, op1=ALU.add)
                nc.vector.tensor_scalar(den[:], th[:], -1.0, 1.0,
                                        op0=ALU.mult, op1=ALU.add)
                nc.vector.reciprocal(den[:], den[:])
                nc.vector.tensor_mul(el[:], el[:], den[:])
                es = cp.tile([BL, 1], F32, tag="es", name="es")
                nc.vector.reduce_sum(es[:], el[:], axis=AX.X)
                einv = cp.tile([BL, 1], F32, tag="einv", name="einv")
                nc.vector.reciprocal(einv[:], es[:])
                rwT = cp.tile([BL, E], F32, tag="rwT", name="rwT")
                nc.vector.tensor_scalar_mul(rwT[:], el[:], einv[:])
                # broadcast rw to all 128 partitions via DRAM bounce
                rw_dram = dp.tile([BL, E], F32, tag="rwd", name="rwd")
                nc.sync.dma_start(rw_dram[:], rwT[:])
                rwd_ap = rw_dram[:]
                bcast_src = bass.AP(
                    tensor=rwd_ap.tensor, offset=rwd_ap.offset,
                    ap=[[0, GP], [1, BL * E]],
                )
                nc.sync.dma_start(rw_bc[:], bcast_src)

                # expert-weight aggregation (runtime routing weights),
                # per-sample so sample b+1's kernel path overlaps sample b's
                # depthwise work: aggregate, fp8-encode 64*kern, lane-shift
                # the bytes into u32
                def emit_agg_kern(b):
                    kv = kern[:, :, b, 0:T]
                    for e in range(E):
                        s_ap = rw_bc[:, E * b + e : E * b + e + 1]
                        if e == 0:
                            nc.vector.tensor_scalar_mul(kv, dwT[:, e], s_ap)
                        else:
                            nc.vector.scalar_tensor_tensor(
                                kv, dwT[:, e], s_ap, kv, op0=ALU.mult, op1=ALU.add)
                    nc.vector.tensor_scalar_mul(kern8[:, :, b], kern[:, :, b],
                                                KSCALE)
                    nc.vector.tensor_scalar_mul(
                        kern32[:, :, b], kern8[:, :, b].bitcast(U8), lane32[:])

                def emit_pwag(b):
                    pv = pwag[:, b]
                    for e in range(E):
                        s_ap = rw_bc[:, E * b + e : E * b + e + 1]
                        if e == 0:
                            nc.vector.tensor_scalar_mul(pv, pwT[:, e], s_ap)
                        else:
                            nc.vector.scalar_tensor_tensor(
                                pv, pwT[:, e], s_ap, pv, op0=ALU.mult, op1=ALU.add)

                def emit_expand(b):
                    # expand conv (fp8 DR over channel halves + ones channel
                    # carrying 8*b1) + silu(psum/8) into padded fp8 layout
                    slot = b % NSLOT
                    for g in range(NG):
                        lhsT = bass.AP(
                            tensor=exp8[:].tensor,
                            offset=exp8[:].offset + g * GP,
                            ap=[exp8[:].ap[0], [HIDP, 2], [1, GP]])
                        for nh in range(NH):
                            pex = ppex.tile([GP, NF], F32, tag="pex", name="pex")
                            x8v = x8[:]
                            rhs = bass.AP(
                                tensor=x8v.tensor,
                                offset=x8v.offset + b * HW + nh * NF,
                                ap=[x8v.ap[0], [BL * HW, 2], [1, NF]])
                            nc.tensor.matmul(pex[:], lhsT, rhs,
                                             start=True, stop=True, perf_mode=DR)
                            xpv = xp_t[slot][g][:, 62 + nh * 420
                                                : 62 + nh * 420 + 418]
                            xpo = bass.AP(tensor=xpv.tensor, offset=xpv.offset,
                                          ap=[xpv.ap[0], [PW, RH], [1, W]])
                            nc.scalar.activation(xpo, pex[:], AF.Silu,
                                                 bias=0.0, scale=1.0 / ESCALE)

                def emit_dw_g(b, g, dgs):
                    # depthwise conv: fp8 DoubleRow tap-pair matmuls, one
                    # stationary per pair shared by both row-halves, one fused
                    # silu per chunk reading both psum banks
                    slot = b % NSLOT
                    pdw = pdwp.tile([GP, NH, 512], F32, tag="pdw", name="pdw")
                    pairs = DW_PAIRS
                    if "half_pairs" in ablate:
                        pairs = DW_PAIRS[:7]
                    dga = dgs[g][:]
                    xpa = xp_t[slot][g][:]
                    for pi, (t0, t1) in enumerate(pairs):
                        stride = 2 if t1 >= T else _tap_off(t1) - _tap_off(t0)
                        lhsT = bass.AP(
                            tensor=dga.tensor, offset=dga.offset + t0 * GP,
                            ap=[dga.ap[0], [(t1 - t0) * GP, 2], [1, GP]])
                        for nh in range(NH):
                            off = 420 * nh + _tap_off(t0)
                            rhs = bass.AP(
                                tensor=xpa.tensor, offset=xpa.offset + off,
                                ap=[xpa.ap[0], [stride, 2], [1, 420]])
                            nc.tensor.matmul(pdw[:, nh, :420], lhsT, rhs,
                                             start=(pi == 0),
                                             stop=(pi == len(pairs) - 1),
                                             perf_mode=DR)
                    pva = pdw[:]
                    pv = bass.AP(tensor=pva.tensor, offset=pva.offset,
                                 ap=[pva.ap[0], [512, NH], [PW, RH], [1, W]])
                    nc.scalar.activation(
                        out2_t[b % NOUT][:, g, :], pv, AF.Silu,
                        bias=b2[:, g : g + 1], scale=1.0 / KSCALE,
                        accum_out=s_parts[b][:, g : g + 1])

                # prime the xp slots so PE has expand work during the routing
                # chain, and sample 0's kernel path so diag(0) fires early
                for b in range(NSLOT):
                    emit_expand(b)
                emit_agg_kern(0)
                emit_pwag(0)
                for b in range(BL):
                    dgs = [dgp.tile([GP, TP, GP], FP8, tag="dg", name="dg")
                           for g in range(NG)]
                    for g in range(NG):
                        build_diag(dgs[g], b, g)
                    emit_dw_g(b, 0, dgs)
                    if b > 0:
                        emit_se1(b - 1)
                    emit_dw_g(b, 1, dgs)
                    if b > 0:
                        emit_se2(b - 1)
                    for g in range(2, NG):
                        emit_dw_g(b, g, dgs)
                    if b > 0:
                        emit_se3(b - 1, (b - 1) % NOUT)
                    if b + NSLOT < BL:
                        emit_expand(b + NSLOT)
                    if b + 1 < BL:
                        emit_agg_kern(b + 1)
                        emit_pwag(b + 1)
                emit_se1(BL - 1)
                emit_se2(BL - 1)
                emit_se3(BL - 1, (BL - 1) % NOUT)

            loop_ctx = (tc.For_i(0, reps, 1, hint_engines=(mybir.EngineType.PE,))
                        if reps > 1 else contextlib.nullcontext())
            with loop_ctx:
                emit_body()

    nc.compile()
    return nc


_NC = None


def _get_nc():
    global _NC
    if _NC is None:
        _NC = _build_program()
    return _NC


def _prep_maps(x, r_w1, r_b1, r_w2, r_b2, exp_w,
               bn1_g, bn1_b, bn1_m, bn1_v, dw_w,
               bn2_g, bn2_b, bn2_m, bn2_v,
               se_w1, se_b1, se_w2, se_b2, pw_w,
               bn3_g, bn3_b, bn3_m, bn3_v):
    f = np.float32
    f8dt = mybir.dt.np(FP8)
    x = np.asarray(x, f).reshape(B, CIN, HW)

    def fold_bn(g, bvec, m, v):
        a = np.asarray(g, f) / np.sqrt(np.asarray(v, f) + EPS)
        return a, np.asarray(bvec, f) - np.asarray(m, f) * a

    a1v, b1v = fold_bn(bn1_g, bn1_b, bn1_m, bn1_v)
    a2v, b2v = fold_bn(bn2_g, bn2_b, bn2_m, bn2_v)
    a3v, b3v = fold_bn(bn3_g, bn3_b, bn3_m, bn3_v)
    # extra /2 absorbs the sigmoid's 0.5 dropped from wsc = pwag*(1+tanh)
    a3v = a3v / np.float32(2.0 * PSCALE)

    def chunk(v):  # [HID] -> [GP, NG] padded
        vp = np.concatenate([np.asarray(v, f), np.zeros(HIDP - HID, f)])
        return vp.reshape(NG, GP).T.copy()

    # expand weights with a1 folded in, x8 scale, + ones channel carrying b1
    expT = np.zeros((CIN, HIDP), f)
    expT[:, :HID] = (np.asarray(exp_w, f) * a1v[:, None]).T
    b1p = np.concatenate([b1v, np.zeros(HIDP - HID, f)])
    exp8 = np.zeros((CH, 2, HIDP), f)
    exp8[: CIN // 2, 0] = expT[: CIN // 2]
    exp8[: CIN // 2, 1] = expT[CIN // 2 :]
    exp8[CIN // 2, 0] = b1p
    exp8 = (exp8 * np.float32(ESCALE)).astype(f8dt)

    # depthwise kernels with a2 folded in (per channel)
    dwf = np.asarray(dw_w, f).reshape(E, HID, T) * a2v[None, :, None]
    dwT = np.zeros((GP, E, NG, T), f)
    pwT = np.zeros((GP, E, NG, COUT), f)
    sw1 = np.zeros((GP, NG, RED), f)
    sw2b = np.zeros((RED, NG, GP), f)
    b2se = np.zeros((GP, NG), f)
    for g in range(NG):
        n = min(GP, HID - g * GP)
        cs = slice(g * GP, g * GP + n)
        dwT[:n, :, g, :] = dwf[:, cs, :].transpose(1, 0, 2)
        pwT[:n, :, g, :] = np.asarray(pw_w, f)[:, :, cs].transpose(2, 0, 1) \
            * np.float32(PSCALE)
        sw1[:n, g, :] = (np.asarray(se_w1, f)[:, cs] / HW).T
        sw2b[:, g, :n] = np.asarray(se_w2, f)[cs, :].T
        b2se[:n, g] = np.asarray(se_b2, f)[cs] / 2

    # u32 diag-build constants: 0/1 mask at u32 elem (t*32 + p//4) and the
    # per-partition byte-lane shift 2^(8*(p%4))
    ident32 = np.zeros((GP, TP * 32), np.uint32)
    for p in range(GP):
        ident32[p, np.arange(TP) * 32 + p // 4] = 1
    lane32 = (1 << (8 * (np.arange(GP) % 4))).astype(f)

    # b3 is pre-added to x (residual carries it); correct the routing bias
    # for the shifted pool: rb1' = rb1 - r_w1 @ b3
    b3c = np.asarray(r_w1, f) @ b3v
    common = dict(
        exp8=exp8,
        b2=chunk(b2v),
        a3=a3v.reshape(COUT, 1),
        dwT=dwT, pwT=pwT, sw1=sw1, sw2b=sw2b, b2se=b2se,
        rw1=(np.asarray(r_w1, f).T / HW).copy(),
        rb1=(np.asarray(r_b1, f) - b3c).reshape(RHID, 1),
        rw2=np.asarray(r_w2, f).T.copy(),
        rb2=np.tile(np.asarray(r_b2, f), (BL, 1)),
        sb1=np.asarray(se_b1, f).reshape(RED, 1),
        ident32=ident32,
        lane32=lane32.reshape(GP, 1),
    )
    xb = x + b3v[None, :, None]
    out = []
    for c in range(NCORES):
        xs = np.ascontiguousarray(x[c * BL : (c + 1) * BL].transpose(1, 0, 2))
        xbs = np.ascontiguousarray(xb[c * BL : (c + 1) * BL].transpose(1, 0, 2))
        x8 = np.zeros((CH, 2, BL, HW), f)
        x8[: CIN // 2, 0] = xs[: CIN // 2]
        x8[: CIN // 2, 1] = xs[CIN // 2 :]
        x8[CIN // 2, 0] = 1.0
        out.append(dict(common, x=xbs, x8=x8.astype(f8dt)))
    return out


def kernel(**inputs):
    from concourse.bass_utils import run_bass_kernel_spmd

    nc = _get_nc()
    in_maps = _prep_maps(**inputs)
    res = run_bass_kernel_spmd(nc, in_maps, core_ids=list(range(NCORES)))
    y = np.concatenate([res.results[c]["y"] for c in range(NCORES)], axis=0)
    return y.reshape(B, COUT, H, W).astype(np.float32)


if __name__ == "__main__":
    t0 = time.time()
    nc = _get_nc()
    print(f"build+compile: {time.time()-t0:.1f}s")
